# revision 8
# baseline (speedup 1.0000x reference)
"""BiLSTM-CRF Trainium2 kernel (nn_BiLSTM_CRF_44435731645126).

Strategy:
  host: gather x = emb[sentence] (avoids shipping the 205MB table) plus pure
        marshaling (transpose/permute/cast/flip) of weights.
  NEFF A (SPMD, cores 0-1): core0 = forward LSTM, core1 = backward LSTM on
        time-reversed input. Per core: Xpre = x@w_ih.T+b GEMM (bf16),
        2048-step recurrence (W_hh stationary on PE, 64 LDW+MM pairs/step),
        partial featsT = w_out_half @ hs GEMM.
  host: flip backward partial feats (marshaling only).
  NEFF B (1 core): featsT_f + featsT_b + b_out -> CRF forward pass as a
        log-semiring scan tree -> logZ scalar.
"""

import os
import numpy as np
import ml_dtypes

import concourse.bass as bass
from concourse import bacc
import concourse.mybir as mybir
import concourse.tile as tile
from concourse.bass import ds, ts
from concourse.bass_utils import run_bass_kernel_spmd
from concourse.masks import make_identity

F32 = mybir.dt.float32
BF16 = mybir.dt.bfloat16
FP8 = mybir.dt.float8e4
AF = mybir.ActivationFunctionType
ALU = mybir.AluOpType

WSCALE = 8.0  # pre-scale on gate weights; undone via activation(scale=1/WSCALE)

T = 2048
E = 512
Hh = 512
G = 2048  # 4*Hh
NT = 5
START, STOP = 3, 4
NEG = -10000.0

U = int(os.environ.get("LSTM_UNROLL", "4"))  # steps per For_i iteration

LAST_INFO = {}

# m-column layout: m = g*4 + b, gate order [i, f, o, g~], b = hidden block.
PERM = np.concatenate([
    np.arange(0, 512),       # i
    np.arange(512, 1024),    # f
    np.arange(1536, 2048),   # o
    np.arange(1024, 1536),   # g~
])


def _to_tiles(mat_t, nk, free):
    """mat_t: [nk*128, free] -> [128, nk, free] with [p, k, f] = mat_t[128k+p, f]."""
    return np.ascontiguousarray(mat_t.reshape(nk, 128, free).transpose(1, 0, 2))


def _new_nc(num_devices):
    return bacc.Bacc("TRN2", target_bir_lowering=False, debug=False,
                     num_devices=num_devices)


def build_lstm_program():
    nc = _new_nc(2)
    NBLOB = 5 * T + 5 * G + 4 * NT + 16
    blob_d = nc.dram_tensor("blob", [128, NBLOB], BF16, kind="ExternalInput")
    whh8_d = nc.dram_tensor("whh8", [128, 4 * G], FP8, kind="ExternalInput")
    featsT_d = nc.dram_tensor("featsT", [NT, T], F32, kind="ExternalOutput")

    mA = [g * 4 + b for g in range(4) for b in range(2)]  # hidden blocks 0,1
    mB = [g * 4 + b for g in range(4) for b in (2, 3)]

    with (
        nc.sbuf_tensor([128, NBLOB], BF16) as blob,
        nc.sbuf_tensor([128, 4 * G], FP8) as whh8sb,
        nc.sbuf_tensor([128, T, 4], BF16) as hs,
    ):
        whhT = whh8sb[:].rearrange("p (k g) -> p k g", k=4)
        o1 = 5 * T + 5 * G
        woutT = blob[:, o1 : o1 + 4 * NT].rearrange("p (k j) -> p k j", k=4)
        o2 = o1 + 4 * NT
        hc0 = blob[:, o2 : o2 + 16].bitcast(F32).rearrange(
            "p (two k) -> p two k", two=2)
        h0 = hc0[:, 0, :]
        c0 = hc0[:, 1, :]
        xT = blob[:, : 5 * T].rearrange("p (k t) -> p k t", k=5)
        wihT = blob[:, 5 * T : 5 * T + 5 * G].rearrange("p (k g) -> p k g", k=5)

        with (
            nc.sbuf_tensor([128, T, 4, 2], BF16) as xpa,
            nc.sbuf_tensor([128, T, 4, 2], BF16) as xpb,
            nc.sbuf_tensor([128, 4], BF16) as hb0,
            nc.sbuf_tensor([128, 4], BF16) as hb1,
            nc.sbuf_tensor([128, 4], F32) as cb0,
            nc.sbuf_tensor([128, 4], F32) as cb1,
        ):
          hbuf = [hb0, hb1]
          cbuf = [cb0, cb1]
          with tile.TileContext(nc) as tc0:
            with tc0.tile_pool(name="psx", bufs=4, space="PSUM") as psx:
                nc.sync.dma_start(blob[:], blob_d[:])
                nc.sync.dma_start(whh8sb[:], whh8_d[:])
                nc.vector.tensor_copy(hbuf[0][:], h0)   # f32 -> bf16 cast
                nc.vector.tensor_copy(cbuf[0][:], c0)

                # ---- phase A: Xpre GEMM ----
                for m in range(16):
                    g_, b_ = divmod(m, 4)
                    for tck in range(4):
                        ps = psx.tile([128, 512], F32, tag="psx")
                        for e in range(5):
                            nc.tensor.matmul(
                                ps[:],
                                wihT[:, e, ts(m, 128)],
                                xT[:, e, ts(tck, 512)],
                                start=(e == 0),
                                stop=(e == 4),
                            )
                        xp_dst = xpa if b_ < 2 else xpb
                        nc.vector.tensor_copy(
                            xp_dst[:, ts(tck, 512), g_, b_ % 2], ps[:]
                        )

          # ---- phase B: recurrence ----
          with tile.TileContext(nc) as tc:
            with (
                tc.tile_pool(name="work", bufs=4) as wpool,
                tc.tile_pool(name="psg", bufs=2, space="PSUM") as psg,
            ):
                with tc.For_i(0, T, U, hint_engines=(mybir.EngineType.PE,),
                              staggered_reset=True) as iv:
                    for u in range(U):
                        toff = nc.snap(iv + u)
                        hcur = hbuf[u % 2]
                        hnxt = hbuf[(u + 1) % 2]
                        ccur = cbuf[u % 2]
                        cnxt = cbuf[(u + 1) % 2]
                        pg = psg.tile([128, 4, 4], F32, tag="pg")
                        for k in (0, 1):
                            for m in mA + mB:
                                g_, b_ = divmod(m, 4)
                                nc.tensor.matmul(
                                    pg[:, g_, b_ : b_ + 1],
                                    whhT[:, k, ts(m, 128)],
                                    hcur[:, k : k + 1],
                                    start=(k == 0),
                                    stop=False,
                                    skip_group_check=True,
                                )
                        for half in (mA, mB):
                            for m in half:
                                g_, b_ = divmod(m, 4)
                                for k in (2, 3):
                                    nc.tensor.matmul(
                                        pg[:, g_, b_ : b_ + 1],
                                        whhT[:, k, ts(m, 128)],
                                        hcur[:, k : k + 1],
                                        start=False,
                                        stop=(k == 3),
                                        skip_group_check=True,
                                    )
                        for bx, xp_src in ((slice(0, 2), xpa), (slice(2, 4), xpb)):
                            ga = wpool.tile([128, 4, 2], F32, tag="ga")
                            nc.vector.tensor_add(
                                ga[:],
                                pg[:, :, bx],
                                xp_src[:, ds(toff, 1), :, :].squeeze(1),
                            )
                            sg = wpool.tile([128, 3, 2], F32, tag="sg")
                            nc.scalar.activation(sg[:], ga[:, 0:3, :], AF.Sigmoid,
                                                 scale=1.0 / WSCALE)
                            tg = wpool.tile([128, 1, 2], F32, tag="tg")
                            nc.scalar.activation(tg[:], ga[:, 3:4, :], AF.Tanh,
                                                 scale=1.0 / WSCALE)
                            ig = wpool.tile([128, 2], F32, tag="ig")
                            nc.vector.tensor_mul(ig[:], sg[:, 0, :], tg[:, 0, :])
                            fc = wpool.tile([128, 2], F32, tag="fc")
                            nc.vector.tensor_mul(fc[:], sg[:, 1, :], ccur[:, bx])
                            nc.vector.tensor_add(cnxt[:, bx], ig[:], fc[:])
                            tc_ = wpool.tile([128, 2], F32, tag="tc")
                            nc.scalar.activation(tc_[:], cnxt[:, bx], AF.Tanh)
                            nc.vector.tensor_mul(hnxt[:, bx], sg[:, 2, :], tc_[:])
                        nc.vector.tensor_copy(
                            hs[:, ds(toff, 1), :].squeeze(1), hnxt[:]
                        )

        # ---- ctx 2: feats GEMM + output (fresh sems: tail drain stays small) ----
        with tile.TileContext(nc) as tc2:
            with (
                tc2.tile_pool(name="fout", bufs=1) as fpool,
                tc2.tile_pool(name="psf", bufs=2, space="PSUM") as psf,
            ):
                fsb = fpool.tile([NT, T], F32, tag="fsb")
                for tck in range(4):
                    pf = psf.tile([NT, 512], F32, tag="pf")
                    for k in range(4):
                        nc.tensor.matmul(
                            pf[:],
                            woutT[:, k, :],
                            hs[:, tck * 512 : (tck + 1) * 512, k],
                            start=(k == 0),
                            stop=(k == 3),
                        )
                    nc.vector.tensor_copy(fsb[:, ts(tck, 512)], pf[:])
                nc.sync.dma_start(featsT_d[:], fsb[:])

    nc.compile()
    return nc


def build_crf_program():
    nc = _new_nc(1)
    ff_d = nc.dram_tensor("ftf", [NT, T], F32, kind="ExternalInput")
    fb_d = nc.dram_tensor("ftb", [NT, T], F32, kind="ExternalInput")
    brep_d = nc.dram_tensor("brep", [128, 16, NT], F32, kind="ExternalInput")
    ta_d = nc.dram_tensor("ta", [128, 125], F32, kind="ExternalInput")
    tb_d = nc.dram_tensor("tb", [128, 125], F32, kind="ExternalInput")
    fv0_d = nc.dram_tensor("fv0r", [1, 25], F32, kind="ExternalInput")
    stp_d = nc.dram_tensor("stpr", [1, 25], F32, kind="ExternalInput")
    out_d = nc.dram_tensor("logz", [1, 1], F32, kind="ExternalOutput")

    with tile.TileContext(nc) as tc:
        with (
            tc.tile_pool(name="c", bufs=1) as cp,
            tc.tile_pool(name="w", bufs=2) as wp,
            tc.tile_pool(name="ps", bufs=2, space="PSUM") as pp,
            tc.tile_pool(name="dr", bufs=1, space="DRAM") as dp,
        ):
            ftf = cp.tile([NT, T], F32)
            nc.sync.dma_start(ftf[:], ff_d[:])
            ftb = cp.tile([NT, T], F32)
            nc.sync.dma_start(ftb[:], fb_d[:])
            brep = cp.tile([128, 16, NT], F32)
            nc.sync.dma_start(brep[:], brep_d[:])
            ta = cp.tile([128, 125], F32)
            nc.sync.dma_start(ta[:], ta_d[:])
            tb = cp.tile([128, 125], F32)
            nc.sync.dma_start(tb[:], tb_d[:])
            fv0r = cp.tile([1, 25], F32)
            nc.sync.dma_start(fv0r[:], fv0_d[:])
            stpr = cp.tile([1, 25], F32)
            nc.sync.dma_start(stpr[:], stp_d[:])

            ident = cp.tile([128, 128], F32, tag="ident")
            make_identity(nc, ident[:])

            # q[p, k, i*5+j] = trans[k,i] + trans[j,k]
            q = cp.tile([128, 5, 25], F32, tag="q")
            nc.vector.tensor_add(
                q[:],
                ta[:].rearrange("p (k x) -> p k x", k=5),
                tb[:].rearrange("p (k x) -> p k x", k=5),
            )

            # F2[p, c, j] = feats[16p + c, j] (both dirs + bias)
            f2 = cp.tile([128, 16, NT], F32, tag="f2")
            for c in range(16):
                pt = pp.tile([128, NT], F32, tag="pt")
                nc.tensor.transpose(pt[:], ftf[:, c::16], ident[0:NT, 0:NT])
                nc.vector.tensor_add(f2[:, c, :], pt[:], brep[:, c, :])
                pt2 = pp.tile([128, NT], F32, tag="pt")
                nc.tensor.transpose(pt2[:], ftb[:, c::16], ident[0:NT, 0:NT])
                nc.vector.tensor_add(f2[:, c, :], f2[:, c, :], pt2[:])

            def lse_k(dst, tsrc, pdim, shape):
                """dst(AP) = logsumexp over innermost k(=5) of tsrc(AP) [pdim, *shape, 5]."""
                mx = wp.tile([pdim] + shape, F32, tag=f"mx{len(shape)}")
                nc.vector.tensor_reduce(mx[:], tsrc, mybir.AxisListType.X, ALU.max)
                mxb = mx[:].unsqueeze(len(shape) + 1).broadcast_to(
                    [pdim] + shape + [5]
                )
                nc.vector.tensor_sub(tsrc, tsrc, mxb)
                nc.scalar.activation(tsrc, tsrc, AF.Exp)
                ssum = wp.tile([pdim] + shape, F32, tag=f"ss{len(shape)}")
                nc.vector.tensor_reduce(ssum[:], tsrc, mybir.AxisListType.X, ALU.add)
                nc.scalar.activation(ssum[:], ssum[:], AF.Ln)
                nc.vector.tensor_add(dst, mx[:], ssum[:])

            # ---- level 0: 2048 A_t -> 1024 products; pair t=(16p+2d, 16p+2d+1) ----
            tstack = wp.tile([128, 8, 25, 5], F32, tag="t0")
            nc.vector.tensor_add(
                tstack[:],
                q[:].rearrange("p k x -> p x k").unsqueeze(1)
                .broadcast_to([128, 8, 25, 5]),
                f2[:, 0::2, :].unsqueeze(2).broadcast_to([128, 8, 25, 5]),
            )
            lvl = cp.tile([128, 8, 25], F32, tag="lvl8")
            lse_k(lvl[:], tstack[:], 128, [8, 25])
            # += f_odd[j] broadcast over i
            nc.vector.tensor_add(
                lvl[:].rearrange("p d (i j) -> p d i j", i=5),
                lvl[:].rearrange("p d (i j) -> p d i j", i=5),
                f2[:, 1::2, :].unsqueeze(2).broadcast_to([128, 8, 5, 5]),
            )

            def pair_level(src, pdim, nd):
                """src[pdim, nd, 25] -> dst[pdim, nd/2, 25]; adjacent pairs.
                tt[p,d,i*5+j,k] = A[p,d,i*5+k] + B[p,d,k*5+j]; built row-by-row
                since DVE APs allow at most 3 free dims."""
                nd2 = nd // 2
                sv = src[:].rearrange("p (d two) x -> p d two x", two=2)
                tt = wp.tile([pdim, nd2, 25, 5], F32, tag=f"tt{nd2}")
                ttv = tt[:].rearrange("p d (i j) k -> p d i j k", i=5)
                bv = (sv[:, :, 1, :].rearrange("p d (k j) -> p d k j", k=5)
                      .rearrange("p d k j -> p d j k"))
                for i in range(5):
                    av = (sv[:, :, 0, i * 5 : (i + 1) * 5]
                          .unsqueeze(2).broadcast_to([pdim, nd2, 5, 5]))
                    nc.vector.tensor_add(ttv[:, :, i, :, :], av, bv)
                dst = cp.tile([pdim, nd2, 25], F32, tag=f"lvl{pdim}_{nd2}")
                lse_k(dst[:], tt[:], pdim, [nd2, 25])
                return dst

            for nd in (8, 4, 2):
                lvl = pair_level(lvl, 128, nd)
            # lvl: [128, 1, 25]

            # repack 8 partitions -> 1 via DRAM roundtrip
            dr1 = dp.tile([128, 25], F32, tag="dr1")
            nc.sync.dma_start(dr1[:], lvl[:].squeeze(1))
            pk = cp.tile([16, 8, 25], F32, tag="pk16")
            nc.sync.dma_start(pk[:], dr1[:].rearrange("(a b) x -> a b x", b=8))
            cur = pk
            for nd in (8, 4, 2):
                cur = pair_level(cur, 16, nd)
            dr2 = dp.tile([16, 25], F32, tag="dr2")
            nc.sync.dma_start(dr2[:], cur[:].squeeze(1))
            pk2 = cp.tile([1, 16, 25], F32, tag="pk2")
            nc.sync.dma_start(pk2[:], dr2[:].rearrange("(a b) x -> a b x", b=16))
            cur = pk2
            for nd in (16, 8, 4, 2):
                cur = pair_level(cur, 1, nd)
            # cur: [1, 1, 25]
            pfin = cp.tile([1, 5, 5], F32, tag="pfin")
            nc.vector.tensor_copy(pfin[:], cur[:].squeeze(1)
                                  .rearrange("p (i j) -> p i j", i=5))
            # logZ = lse over 25 of (fv0[i] + P[i,j] + trans[STOP, j])
            pfl = pfin[:].rearrange("p i j -> p (i j)")
            nc.vector.tensor_add(pfl, pfl, fv0r[:])
            nc.vector.tensor_add(pfl, pfl, stpr[:])
            m2 = wp.tile([1, 1], F32, tag="m2")
            nc.vector.tensor_reduce(m2[:], pfl, mybir.AxisListType.X, ALU.max)
            nc.vector.tensor_sub(pfl, pfl, m2[:].broadcast_to([1, 25]))
            nc.scalar.activation(pfl, pfl, AF.Exp)
            s2 = wp.tile([1, 1], F32, tag="s2")
            nc.vector.tensor_reduce(s2[:], pfl, mybir.AxisListType.X, ALU.add)
            nc.scalar.activation(s2[:], s2[:], AF.Ln)
            res = cp.tile([1, 1], F32, tag="res")
            nc.vector.tensor_add(res[:], s2[:], m2[:])
            nc.sync.dma_start(out_d[:], res[:])

    nc.compile()
    return nc


def _prep_dir(x, w_ih, w_hh, b, h0d, c0d, w_out_half):
    bf = ml_dtypes.bfloat16
    xT = _to_tiles(np.concatenate(
        [np.ascontiguousarray(x.T), np.ones((1, T), np.float32),
         np.zeros((127, T), np.float32)], 0), 5, T).astype(bf)
    wihT = _to_tiles(np.concatenate(
        [np.ascontiguousarray(WSCALE * w_ih[PERM].T),
         WSCALE * b[PERM][None, :].astype(np.float32),
         np.zeros((127, G), np.float32)], 0), 5, G).astype(bf)
    whh8 = _to_tiles(np.ascontiguousarray(WSCALE * w_hh[PERM].T), 4, G).astype(
        ml_dtypes.float8_e4m3)
    woutT = _to_tiles(np.ascontiguousarray(w_out_half.T), 4, NT).astype(bf)
    hc0 = np.stack([h0d.reshape(4, 128).T, c0d.reshape(4, 128).T], 1)
    hc0_bits = np.ascontiguousarray(hc0.astype(np.float32)).view(np.uint16)
    blob = np.concatenate(
        [xT.reshape(128, -1).view(np.uint16),
         wihT.reshape(128, -1).view(np.uint16),
         woutT.reshape(128, -1).view(np.uint16),
         hc0_bits.reshape(128, 16)], 1)
    return dict(blob=np.ascontiguousarray(blob).view(bf),
                whh8=np.ascontiguousarray(whh8.reshape(128, -1)))


def kernel(sentence, emb, w_ih_f, w_hh_f, b_f, w_ih_b, w_hh_b, b_b,
           w_out, b_out, transitions, h0, c0):
    sentence = np.asarray(sentence)
    emb = np.asarray(emb, dtype=np.float32)
    x = emb[sentence.astype(np.int64)]  # [T, E] host gather
    h0 = np.asarray(h0, np.float32)
    c0 = np.asarray(c0, np.float32)
    w_out = np.asarray(w_out, np.float32)

    in_f = _prep_dir(x, np.asarray(w_ih_f, np.float32),
                     np.asarray(w_hh_f, np.float32), np.asarray(b_f, np.float32),
                     h0[0, 0], c0[0, 0], w_out[:, :Hh])
    in_b = _prep_dir(x[::-1], np.asarray(w_ih_b, np.float32),
                     np.asarray(w_hh_b, np.float32), np.asarray(b_b, np.float32),
                     h0[1, 0], c0[1, 0], w_out[:, Hh:])

    nc_a = build_lstm_program()
    res_a = run_bass_kernel_spmd(nc_a, [in_f, in_b], core_ids=[0, 1])
    ftf = res_a.results[0]["featsT"]           # [5, T]
    ftb = res_a.results[1]["featsT"][:, ::-1]  # un-reverse (marshaling)
    LAST_INFO["neff_a_ns"] = res_a.exec_time_ns
    if res_a.instructions_and_trace:
        LAST_INFO["trace_a"] = res_a.instructions_and_trace[1]

    trans = np.asarray(transitions, np.float32)
    b_out = np.asarray(b_out, np.float32)
    k_, i_, j_ = np.meshgrid(np.arange(5), np.arange(5), np.arange(5), indexing="ij")
    ta = trans[k_, i_]  # [k,i,j] = trans[k,i]
    tb = trans[j_, k_]  # [k,i,j] = trans[j,k]
    ta_rep = np.ascontiguousarray(
        np.broadcast_to(ta.reshape(1, 125), (128, 125))).astype(np.float32)
    tb_rep = np.ascontiguousarray(
        np.broadcast_to(tb.reshape(1, 125), (128, 125))).astype(np.float32)
    brep = np.ascontiguousarray(
        np.broadcast_to(b_out[None, None, :], (128, 16, 5))).astype(np.float32)
    fv0 = np.full((NT,), NEG, np.float32)
    fv0[START] = 0.0
    fv0_rep = np.ascontiguousarray(np.repeat(fv0, 5)[None, :]).astype(np.float32)
    stp_rep = np.ascontiguousarray(np.tile(trans[STOP], 5)[None, :]).astype(np.float32)

    nc_b = build_crf_program()
    in_crf = dict(ftf=np.ascontiguousarray(ftf).astype(np.float32),
                  ftb=np.ascontiguousarray(ftb).astype(np.float32),
                  brep=brep, ta=ta_rep, tb=tb_rep, fv0r=fv0_rep, stpr=stp_rep)
    res_b = run_bass_kernel_spmd(nc_b, [in_crf], core_ids=[0])
    LAST_INFO["neff_b_ns"] = res_b.exec_time_ns
    if res_b.instructions_and_trace:
        LAST_INFO["trace_b"] = res_b.instructions_and_trace[1]
    out = res_b.results[0]["logz"].reshape(())
    return np.asarray(out, dtype=np.float32).reshape(())



# revision 9
# speedup vs baseline: 1.3018x; 1.3018x over previous
"""BiLSTM-CRF Trainium2 kernel (nn_BiLSTM_CRF_44435731645126).

Strategy:
  host: gather x = emb[sentence] (avoids shipping the 205MB table) plus pure
        marshaling (transpose/permute/cast/flip) of weights.
  NEFF A (SPMD, cores 0-1): core0 = forward LSTM, core1 = backward LSTM on
        time-reversed input. Per core: Xpre = x@w_ih.T+b GEMM (bf16),
        2048-step recurrence (64 LDW+MM pairs/step at the ~27ns MM dispatch
        floor), partial featsT = w_out_half @ hs GEMM.
  host: flip backward partial feats (marshaling only).
  NEFF B (1 core): featsT_f + featsT_b + b_out -> CRF forward pass as a
        log-semiring scan tree -> logZ scalar.

Recurrence critical path design (v3):
  - per-half PSUM gate tiles pgA/pgB: half-A tail starts after its 32 MMs.
  - Xpre injected into PSUM by one identity-stationary matmul per half
    (rhs = xp[:, t] dynamic slice), so ACT reads gates straight from PSUM.
  - tanh(g~) computed as 2*sigmoid(2x)-1: g~ rows of W_hh/W_ih/b are
    pre-scaled by 2 on host; ONE sigmoid covers all 4 gates of a half.
    Reconstruction is fused into scalar_tensor_tensor ops:
      q  = (sig_g - 0.5) * sig_i          # = i*tanh(g)/2
      c' = (q * 2) + f*c
  - f*c and the hs spill run on GpSimd (otherwise idle).
"""

import os
import numpy as np
import ml_dtypes

import concourse.bass as bass
from concourse import bacc
import concourse.mybir as mybir
import concourse.tile as tile
from concourse.bass import ds, ts
from concourse.bass_utils import run_bass_kernel_spmd
from concourse.masks import make_identity

F32 = mybir.dt.float32
BF16 = mybir.dt.bfloat16
AF = mybir.ActivationFunctionType
ALU = mybir.AluOpType

T = 2048
E = 512
Hh = 512
G = 2048  # 4*Hh
NT = 5
START, STOP = 3, 4
NEG = -10000.0

U = int(os.environ.get("LSTM_UNROLL", "8"))  # steps per For_i iteration

LAST_INFO = {}

# m-column layout: m = g*4 + b, gate order [i, f, o, g~], b = hidden block.
PERM = np.concatenate([
    np.arange(0, 512),       # i
    np.arange(512, 1024),    # f
    np.arange(1536, 2048),   # o
    np.arange(1024, 1536),   # g~
])


def _to_tiles(mat_t, nk, free):
    """mat_t: [nk*128, free] -> [128, nk, free] with [p, k, f] = mat_t[128k+p, f]."""
    return np.ascontiguousarray(mat_t.reshape(nk, 128, free).transpose(1, 0, 2))


def _new_nc(num_devices):
    return bacc.Bacc("TRN2", target_bir_lowering=False, debug=False,
                     num_devices=num_devices)


def build_lstm_program():
    nc = _new_nc(2)
    NBLOB = 5 * T + 5 * G + 4 * G + 4 * NT + 16
    blob_d = nc.dram_tensor("blob", [128, NBLOB], BF16, kind="ExternalInput")
    featsT_d = nc.dram_tensor("featsT", [NT, T], F32, kind="ExternalOutput")

    mA = [g * 4 + b for g in range(4) for b in range(2)]  # hidden blocks 0,1
    mB = [g * 4 + b for g in range(4) for b in (2, 3)]

    with (
        nc.sbuf_tensor([128, NBLOB], BF16) as blob,
        nc.sbuf_tensor([128, T, 4], BF16) as hs,
    ):
        o0 = 5 * T + 5 * G
        whhT = blob[:, o0 : o0 + 4 * G].rearrange("p (k g) -> p k g", k=4)
        o1 = o0 + 4 * G
        woutT = blob[:, o1 : o1 + 4 * NT].rearrange("p (k j) -> p k j", k=4)
        o2 = o1 + 4 * NT
        hc0 = blob[:, o2 : o2 + 16].bitcast(F32).rearrange(
            "p (two k) -> p two k", two=2)
        h0 = hc0[:, 0, :]
        c0 = hc0[:, 1, :]
        xT = blob[:, : 5 * T].rearrange("p (k t) -> p k t", k=5)
        wihT = blob[:, 5 * T : 5 * T + 5 * G].rearrange("p (k g) -> p k g", k=5)

        with (
            nc.sbuf_tensor([128, T, 4, 4], BF16) as xp,
            nc.sbuf_tensor([128, 128], BF16) as ident,
            nc.sbuf_tensor([128, 4], BF16) as hb0,
            nc.sbuf_tensor([128, 4], BF16) as hb1,
            nc.sbuf_tensor([128, 4], F32) as cb0,
            nc.sbuf_tensor([128, 4], F32) as cb1,
        ):
          hbuf = [hb0, hb1]
          cbuf = [cb0, cb1]
          with tile.TileContext(nc) as tc0:
            with tc0.tile_pool(name="psx", bufs=4, space="PSUM") as psx:
                nc.sync.dma_start(blob[:], blob_d[:])
                make_identity(nc, ident[:])
                nc.vector.tensor_copy(hbuf[0][:], h0)   # f32 -> bf16 cast
                nc.vector.tensor_copy(cbuf[0][:], c0)

                # ---- phase A: Xpre GEMM ----
                for m in range(16):
                    g_, b_ = divmod(m, 4)
                    for tck in range(4):
                        ps = psx.tile([128, 512], F32, tag="psx")
                        for e in range(5):
                            nc.tensor.matmul(
                                ps[:],
                                wihT[:, e, ts(m, 128)],
                                xT[:, e, ts(tck, 512)],
                                start=(e == 0),
                                stop=(e == 4),
                            )
                        nc.vector.tensor_copy(
                            xp[:, ts(tck, 512), g_, b_], ps[:]
                        )

          # ---- phase B: recurrence ----
          with tile.TileContext(nc) as tc:
            with (
                tc.tile_pool(name="work", bufs=4) as wpool,
                tc.tile_pool(name="psg", bufs=4, space="PSUM") as psg,
            ):
                with tc.For_i(0, T, U, hint_engines=(mybir.EngineType.PE,),
                              staggered_reset=True) as iv:
                    for u in range(U):
                        toff = nc.snap(iv + u)
                        hcur = hbuf[u % 2]
                        hnxt = hbuf[(u + 1) % 2]
                        ccur = cbuf[u % 2]
                        cnxt = cbuf[(u + 1) % 2]
                        pgA = psg.tile([128, 4, 2], F32, tag="pgA")
                        pgB = psg.tile([128, 4, 2], F32, tag="pgB")
                        # k=0,1 for all m (needs h blocks 0,1 = half A of t-1)
                        for k in (0, 1):
                            for m in mA + mB:
                                g_, b_ = divmod(m, 4)
                                pg = pgA if b_ < 2 else pgB
                                nc.tensor.matmul(
                                    pg[:, g_, (b_ % 2) : (b_ % 2) + 1],
                                    whhT[:, k, ts(m, 128)],
                                    hcur[:, k : k + 1],
                                    start=(k == 0),
                                    stop=False,
                                    skip_group_check=True,
                                )
                        # half A: k=2,3 + Xpre inject; then half B
                        for half, pg, bx in ((mA, pgA, slice(0, 2)),
                                             (mB, pgB, slice(2, 4))):
                            nc.tensor.matmul(
                                pg[:, :, :],
                                ident[:],
                                xp[:, ds(toff, 1), :, bx].squeeze(1),
                                start=False,
                                stop=False,
                                skip_group_check=True,
                            )
                            for m in half:
                                g_, b_ = divmod(m, 4)
                                for k in (2, 3):
                                    nc.tensor.matmul(
                                        pg[:, g_, (b_ % 2) : (b_ % 2) + 1],
                                        whhT[:, k, ts(m, 128)],
                                        hcur[:, k : k + 1],
                                        start=False,
                                        stop=(k == 3 and m == half[-1]),
                                        skip_group_check=True,
                                    )
                        for pg, bx in ((pgA, slice(0, 2)), (pgB, slice(2, 4))):
                            # sg rows: 0=sig(i) 1=sig(f) 2=sig(o) 3=sig(2g~)
                            sg = wpool.tile([128, 4, 2], F32, tag="sg")
                            nc.scalar.activation(sg[:], pg[:], AF.Sigmoid)
                            fc = wpool.tile([128, 2], F32, tag="fc")
                            nc.gpsimd.tensor_mul(fc[:], sg[:, 1, :], ccur[:, bx])
                            q = wpool.tile([128, 2], F32, tag="q")
                            # q = (sig_g - 0.5) * sig_i = i*tanh(g~)/2
                            nc.vector.scalar_tensor_tensor(
                                q[:], sg[:, 3, :], 0.5, sg[:, 0, :],
                                ALU.subtract, ALU.mult)
                            # c' = q*2 + fc
                            nc.vector.scalar_tensor_tensor(
                                cnxt[:, bx], q[:], 2.0, fc[:],
                                ALU.mult, ALU.add)
                            tc_ = wpool.tile([128, 2], F32, tag="tc")
                            nc.scalar.activation(tc_[:], cnxt[:, bx], AF.Tanh)
                            nc.vector.tensor_mul(hnxt[:, bx], sg[:, 2, :], tc_[:])
                        nc.gpsimd.tensor_copy(
                            hs[:, ds(toff, 1), :].squeeze(1), hnxt[:]
                        )

        # ---- ctx 2: feats GEMM + output (fresh sems: tail drain stays small) ----
        with tile.TileContext(nc) as tc2:
            with (
                tc2.tile_pool(name="fout", bufs=1) as fpool,
                tc2.tile_pool(name="psf", bufs=2, space="PSUM") as psf,
            ):
                fsb = fpool.tile([NT, T], F32, tag="fsb")
                for tck in range(4):
                    pf = psf.tile([NT, 512], F32, tag="pf")
                    for k in range(4):
                        nc.tensor.matmul(
                            pf[:],
                            woutT[:, k, :],
                            hs[:, tck * 512 : (tck + 1) * 512, k],
                            start=(k == 0),
                            stop=(k == 3),
                        )
                    nc.vector.tensor_copy(fsb[:, ts(tck, 512)], pf[:])
                nc.sync.dma_start(featsT_d[:], fsb[:])

    nc.compile()
    return nc


def build_crf_program():
    nc = _new_nc(1)
    ff_d = nc.dram_tensor("ftf", [NT, T], F32, kind="ExternalInput")
    fb_d = nc.dram_tensor("ftb", [NT, T], F32, kind="ExternalInput")
    brep_d = nc.dram_tensor("brep", [128, 16, NT], F32, kind="ExternalInput")
    ta_d = nc.dram_tensor("ta", [128, 125], F32, kind="ExternalInput")
    tb_d = nc.dram_tensor("tb", [128, 125], F32, kind="ExternalInput")
    fv0_d = nc.dram_tensor("fv0r", [1, 25], F32, kind="ExternalInput")
    stp_d = nc.dram_tensor("stpr", [1, 25], F32, kind="ExternalInput")
    out_d = nc.dram_tensor("logz", [1, 1], F32, kind="ExternalOutput")

    with tile.TileContext(nc) as tc:
        with (
            tc.tile_pool(name="c", bufs=1) as cp,
            tc.tile_pool(name="w", bufs=2) as wp,
            tc.tile_pool(name="ps", bufs=2, space="PSUM") as pp,
            tc.tile_pool(name="dr", bufs=1, space="DRAM") as dp,
        ):
            ftf = cp.tile([NT, T], F32)
            nc.sync.dma_start(ftf[:], ff_d[:])
            ftb = cp.tile([NT, T], F32)
            nc.sync.dma_start(ftb[:], fb_d[:])
            brep = cp.tile([128, 16, NT], F32)
            nc.sync.dma_start(brep[:], brep_d[:])
            ta = cp.tile([128, 125], F32)
            nc.sync.dma_start(ta[:], ta_d[:])
            tb = cp.tile([128, 125], F32)
            nc.sync.dma_start(tb[:], tb_d[:])
            fv0r = cp.tile([1, 25], F32)
            nc.sync.dma_start(fv0r[:], fv0_d[:])
            stpr = cp.tile([1, 25], F32)
            nc.sync.dma_start(stpr[:], stp_d[:])

            ident = cp.tile([128, 128], F32, tag="ident")
            make_identity(nc, ident[:])

            # q[p, k, i*5+j] = trans[k,i] + trans[j,k]
            q = cp.tile([128, 5, 25], F32, tag="q")
            nc.vector.tensor_add(
                q[:],
                ta[:].rearrange("p (k x) -> p k x", k=5),
                tb[:].rearrange("p (k x) -> p k x", k=5),
            )

            # F2[p, c, j] = feats[16p + c, j] (both dirs + bias)
            f2 = cp.tile([128, 16, NT], F32, tag="f2")
            for c in range(16):
                pt = pp.tile([128, NT], F32, tag="pt")
                nc.tensor.transpose(pt[:], ftf[:, c::16], ident[0:NT, 0:NT])
                nc.vector.tensor_add(f2[:, c, :], pt[:], brep[:, c, :])
                pt2 = pp.tile([128, NT], F32, tag="pt")
                nc.tensor.transpose(pt2[:], ftb[:, c::16], ident[0:NT, 0:NT])
                nc.vector.tensor_add(f2[:, c, :], f2[:, c, :], pt2[:])

            def lse_k(dst, tsrc, pdim, shape):
                """dst(AP) = logsumexp over innermost k(=5) of tsrc(AP) [pdim, *shape, 5]."""
                mx = wp.tile([pdim] + shape, F32, tag=f"mx{len(shape)}")
                nc.vector.tensor_reduce(mx[:], tsrc, mybir.AxisListType.X, ALU.max)
                mxb = mx[:].unsqueeze(len(shape) + 1).broadcast_to(
                    [pdim] + shape + [5]
                )
                nc.vector.tensor_sub(tsrc, tsrc, mxb)
                nc.scalar.activation(tsrc, tsrc, AF.Exp)
                ssum = wp.tile([pdim] + shape, F32, tag=f"ss{len(shape)}")
                nc.vector.tensor_reduce(ssum[:], tsrc, mybir.AxisListType.X, ALU.add)
                nc.scalar.activation(ssum[:], ssum[:], AF.Ln)
                nc.vector.tensor_add(dst, mx[:], ssum[:])

            # ---- level 0: 2048 A_t -> 1024 products; pair t=(16p+2d, 16p+2d+1) ----
            tstack = wp.tile([128, 8, 25, 5], F32, tag="t0")
            nc.vector.tensor_add(
                tstack[:],
                q[:].rearrange("p k x -> p x k").unsqueeze(1)
                .broadcast_to([128, 8, 25, 5]),
                f2[:, 0::2, :].unsqueeze(2).broadcast_to([128, 8, 25, 5]),
            )
            lvl = cp.tile([128, 8, 25], F32, tag="lvl8")
            lse_k(lvl[:], tstack[:], 128, [8, 25])
            # += f_odd[j] broadcast over i
            nc.vector.tensor_add(
                lvl[:].rearrange("p d (i j) -> p d i j", i=5),
                lvl[:].rearrange("p d (i j) -> p d i j", i=5),
                f2[:, 1::2, :].unsqueeze(2).broadcast_to([128, 8, 5, 5]),
            )

            def pair_level(src, pdim, nd):
                """src[pdim, nd, 25] -> dst[pdim, nd/2, 25]; adjacent pairs.
                tt[p,d,i*5+j,k] = A[p,d,i*5+k] + B[p,d,k*5+j]; built row-by-row
                since DVE APs allow at most 3 free dims."""
                nd2 = nd // 2
                sv = src[:].rearrange("p (d two) x -> p d two x", two=2)
                tt = wp.tile([pdim, nd2, 25, 5], F32, tag=f"tt{nd2}")
                ttv = tt[:].rearrange("p d (i j) k -> p d i j k", i=5)
                bv = (sv[:, :, 1, :].rearrange("p d (k j) -> p d k j", k=5)
                      .rearrange("p d k j -> p d j k"))
                for i in range(5):
                    av = (sv[:, :, 0, i * 5 : (i + 1) * 5]
                          .unsqueeze(2).broadcast_to([pdim, nd2, 5, 5]))
                    nc.vector.tensor_add(ttv[:, :, i, :, :], av, bv)
                dst = cp.tile([pdim, nd2, 25], F32, tag=f"lvl{pdim}_{nd2}")
                lse_k(dst[:], tt[:], pdim, [nd2, 25])
                return dst

            for nd in (8, 4, 2):
                lvl = pair_level(lvl, 128, nd)
            # lvl: [128, 1, 25]

            # repack 8 partitions -> 1 via DRAM roundtrip
            dr1 = dp.tile([128, 25], F32, tag="dr1")
            nc.sync.dma_start(dr1[:], lvl[:].squeeze(1))
            pk = cp.tile([16, 8, 25], F32, tag="pk16")
            nc.sync.dma_start(pk[:], dr1[:].rearrange("(a b) x -> a b x", b=8))
            cur = pk
            for nd in (8, 4, 2):
                cur = pair_level(cur, 16, nd)
            dr2 = dp.tile([16, 25], F32, tag="dr2")
            nc.sync.dma_start(dr2[:], cur[:].squeeze(1))
            pk2 = cp.tile([1, 16, 25], F32, tag="pk2")
            nc.sync.dma_start(pk2[:], dr2[:].rearrange("(a b) x -> a b x", b=16))
            cur = pk2
            for nd in (16, 8, 4, 2):
                cur = pair_level(cur, 1, nd)
            # cur: [1, 1, 25]
            pfin = cp.tile([1, 5, 5], F32, tag="pfin")
            nc.vector.tensor_copy(pfin[:], cur[:].squeeze(1)
                                  .rearrange("p (i j) -> p i j", i=5))
            # logZ = lse over 25 of (fv0[i] + P[i,j] + trans[STOP, j])
            pfl = pfin[:].rearrange("p i j -> p (i j)")
            nc.vector.tensor_add(pfl, pfl, fv0r[:])
            nc.vector.tensor_add(pfl, pfl, stpr[:])
            m2 = wp.tile([1, 1], F32, tag="m2")
            nc.vector.tensor_reduce(m2[:], pfl, mybir.AxisListType.X, ALU.max)
            nc.vector.tensor_sub(pfl, pfl, m2[:].broadcast_to([1, 25]))
            nc.scalar.activation(pfl, pfl, AF.Exp)
            s2 = wp.tile([1, 1], F32, tag="s2")
            nc.vector.tensor_reduce(s2[:], pfl, mybir.AxisListType.X, ALU.add)
            nc.scalar.activation(s2[:], s2[:], AF.Ln)
            res = cp.tile([1, 1], F32, tag="res")
            nc.vector.tensor_add(res[:], s2[:], m2[:])
            nc.sync.dma_start(out_d[:], res[:])

    nc.compile()
    return nc


def _prep_dir(x, w_ih, w_hh, b, h0d, c0d, w_out_half):
    bf = ml_dtypes.bfloat16
    # gate order after PERM: [i, f, o, g~]; g~ rows (last Hh) pre-scaled by 2
    # so tanh(z) = 2*sigmoid(2z)-1 needs only a single sigmoid table hit.
    gsc = np.ones((G, 1), np.float32)
    gsc[3 * Hh :] = 2.0
    xT = _to_tiles(np.concatenate(
        [np.ascontiguousarray(x.T), np.ones((1, T), np.float32),
         np.zeros((127, T), np.float32)], 0), 5, T).astype(bf)
    wihT = _to_tiles(np.concatenate(
        [np.ascontiguousarray((gsc * w_ih[PERM]).T),
         (gsc[:, 0] * b[PERM])[None, :].astype(np.float32),
         np.zeros((127, G), np.float32)], 0), 5, G).astype(bf)
    whhT = _to_tiles(np.ascontiguousarray((gsc * w_hh[PERM]).T), 4, G).astype(bf)
    woutT = _to_tiles(np.ascontiguousarray(w_out_half.T), 4, NT).astype(bf)
    hc0 = np.stack([h0d.reshape(4, 128).T, c0d.reshape(4, 128).T], 1)
    hc0_bits = np.ascontiguousarray(hc0.astype(np.float32)).view(np.uint16)
    blob = np.concatenate(
        [xT.reshape(128, -1).view(np.uint16),
         wihT.reshape(128, -1).view(np.uint16),
         whhT.reshape(128, -1).view(np.uint16),
         woutT.reshape(128, -1).view(np.uint16),
         hc0_bits.reshape(128, 16)], 1)
    return dict(blob=np.ascontiguousarray(blob).view(bf))


def kernel(sentence, emb, w_ih_f, w_hh_f, b_f, w_ih_b, w_hh_b, b_b,
           w_out, b_out, transitions, h0, c0):
    sentence = np.asarray(sentence)
    emb = np.asarray(emb, dtype=np.float32)
    x = emb[sentence.astype(np.int64)]  # [T, E] host gather
    h0 = np.asarray(h0, np.float32)
    c0 = np.asarray(c0, np.float32)
    w_out = np.asarray(w_out, np.float32)

    in_f = _prep_dir(x, np.asarray(w_ih_f, np.float32),
                     np.asarray(w_hh_f, np.float32), np.asarray(b_f, np.float32),
                     h0[0, 0], c0[0, 0], w_out[:, :Hh])
    in_b = _prep_dir(x[::-1], np.asarray(w_ih_b, np.float32),
                     np.asarray(w_hh_b, np.float32), np.asarray(b_b, np.float32),
                     h0[1, 0], c0[1, 0], w_out[:, Hh:])

    nc_a = build_lstm_program()
    res_a = run_bass_kernel_spmd(nc_a, [in_f, in_b], core_ids=[0, 1])
    ftf = res_a.results[0]["featsT"]           # [5, T]
    ftb = res_a.results[1]["featsT"][:, ::-1]  # un-reverse (marshaling)
    LAST_INFO["neff_a_ns"] = res_a.exec_time_ns
    if res_a.instructions_and_trace:
        LAST_INFO["trace_a"] = res_a.instructions_and_trace[1]

    trans = np.asarray(transitions, np.float32)
    b_out = np.asarray(b_out, np.float32)
    k_, i_, j_ = np.meshgrid(np.arange(5), np.arange(5), np.arange(5), indexing="ij")
    ta = trans[k_, i_]  # [k,i,j] = trans[k,i]
    tb = trans[j_, k_]  # [k,i,j] = trans[j,k]
    ta_rep = np.ascontiguousarray(
        np.broadcast_to(ta.reshape(1, 125), (128, 125))).astype(np.float32)
    tb_rep = np.ascontiguousarray(
        np.broadcast_to(tb.reshape(1, 125), (128, 125))).astype(np.float32)
    brep = np.ascontiguousarray(
        np.broadcast_to(b_out[None, None, :], (128, 16, 5))).astype(np.float32)
    fv0 = np.full((NT,), NEG, np.float32)
    fv0[START] = 0.0
    fv0_rep = np.ascontiguousarray(np.repeat(fv0, 5)[None, :]).astype(np.float32)
    stp_rep = np.ascontiguousarray(np.tile(trans[STOP], 5)[None, :]).astype(np.float32)

    nc_b = build_crf_program()
    in_crf = dict(ftf=np.ascontiguousarray(ftf).astype(np.float32),
                  ftb=np.ascontiguousarray(ftb).astype(np.float32),
                  brep=brep, ta=ta_rep, tb=tb_rep, fv0r=fv0_rep, stpr=stp_rep)
    res_b = run_bass_kernel_spmd(nc_b, [in_crf], core_ids=[0])
    LAST_INFO["neff_b_ns"] = res_b.exec_time_ns
    if res_b.instructions_and_trace:
        LAST_INFO["trace_b"] = res_b.instructions_and_trace[1]
    out = res_b.results[0]["logz"].reshape(())
    return np.asarray(out, dtype=np.float32).reshape(())


# revision 11
# speedup vs baseline: 1.3027x; 1.0007x over previous
"""BiLSTM-CRF Trainium2 kernel (nn_BiLSTM_CRF_44435731645126).

Strategy:
  host: gather x = emb[sentence] (avoids shipping the 205MB table) plus pure
        marshaling (transpose/permute/cast/flip) of weights.
  NEFF A (SPMD, cores 0-1): core0 = forward LSTM, core1 = backward LSTM on
        time-reversed input. Per core: Xpre = x@w_ih.T+b GEMM (bf16),
        2048-step recurrence (64 LDW+MM pairs/step at the ~27ns MM dispatch
        floor), partial featsT = w_out_half @ hs GEMM.
  host: flip backward partial feats (marshaling only).
  NEFF B (1 core): featsT_f + featsT_b + b_out -> CRF forward pass as a
        log-semiring scan tree -> logZ scalar.

Recurrence critical path design (v3):
  - per-half PSUM gate tiles pgA/pgB: half-A tail starts after its 32 MMs.
  - Xpre injected into PSUM by one identity-stationary matmul per half
    (rhs = xp[:, t] dynamic slice), so ACT reads gates straight from PSUM.
  - tanh(g~) computed as 2*sigmoid(2x)-1: g~ rows of W_hh/W_ih/b are
    pre-scaled by 2 on host; ONE sigmoid covers all 4 gates of a half.
    Reconstruction is fused into scalar_tensor_tensor ops:
      q  = (sig_g - 0.5) * sig_i          # = i*tanh(g)/2
      c' = (q * 2) + f*c
  - f*c and the hs spill run on GpSimd (otherwise idle).
"""

import os
import numpy as np
import ml_dtypes

import concourse.bass as bass
from concourse import bacc
import concourse.mybir as mybir
import concourse.tile as tile
from concourse.bass import ds, ts
from concourse.bass_utils import run_bass_kernel_spmd
from concourse.masks import make_identity

F32 = mybir.dt.float32
BF16 = mybir.dt.bfloat16
AF = mybir.ActivationFunctionType
ALU = mybir.AluOpType

T = 2048
E = 512
Hh = 512
G = 2048  # 4*Hh
NT = 5
START, STOP = 3, 4
NEG = -10000.0

U = int(os.environ.get("LSTM_UNROLL", "8"))  # steps per For_i iteration

LAST_INFO = {}

# m-column layout: m = g*4 + b, gate order [i, f, o, g~], b = hidden block.
PERM = np.concatenate([
    np.arange(0, 512),       # i
    np.arange(512, 1024),    # f
    np.arange(1536, 2048),   # o
    np.arange(1024, 1536),   # g~
])


def _to_tiles(mat_t, nk, free):
    """mat_t: [nk*128, free] -> [128, nk, free] with [p, k, f] = mat_t[128k+p, f]."""
    return np.ascontiguousarray(mat_t.reshape(nk, 128, free).transpose(1, 0, 2))


def _new_nc(num_devices):
    return bacc.Bacc("TRN2", target_bir_lowering=False, debug=False,
                     num_devices=num_devices)


def build_lstm_program():
    nc = _new_nc(2)
    NBLOB = 5 * T + 5 * G + 4 * G + 4 * NT + 16
    blob_d = nc.dram_tensor("blob", [128, NBLOB], BF16, kind="ExternalInput")
    featsT_d = nc.dram_tensor("featsT", [NT, T], F32, kind="ExternalOutput")

    mA = [g * 4 + b for g in range(4) for b in range(2)]  # hidden blocks 0,1
    mB = [g * 4 + b for g in range(4) for b in (2, 3)]

    with (
        nc.sbuf_tensor([128, NBLOB], BF16) as blob,
        nc.sbuf_tensor([128, T, 4], BF16) as hs,
    ):
        o0 = 5 * T + 5 * G
        whhT = blob[:, o0 : o0 + 4 * G].rearrange("p (k g) -> p k g", k=4)
        o1 = o0 + 4 * G
        woutT = blob[:, o1 : o1 + 4 * NT].rearrange("p (k j) -> p k j", k=4)
        o2 = o1 + 4 * NT
        hc0 = blob[:, o2 : o2 + 16].bitcast(F32).rearrange(
            "p (two k) -> p two k", two=2)
        h0 = hc0[:, 0, :]
        c0 = hc0[:, 1, :]
        xT = blob[:, : 5 * T].rearrange("p (k t) -> p k t", k=5)
        wihT = blob[:, 5 * T : 5 * T + 5 * G].rearrange("p (k g) -> p k g", k=5)

        with (
            nc.sbuf_tensor([128, T, 4, 4], BF16) as xp,
            nc.sbuf_tensor([128, 128], BF16) as ident,
            nc.sbuf_tensor([128, 4], BF16) as hb0,
            nc.sbuf_tensor([128, 4], BF16) as hb1,
            nc.sbuf_tensor([128, 4], F32) as cb0,
            nc.sbuf_tensor([128, 4], F32) as cb1,
        ):
          hbuf = [hb0, hb1]
          cbuf = [cb0, cb1]
          with tile.TileContext(nc) as tc0:
            with tc0.tile_pool(name="psx", bufs=4, space="PSUM") as psx:
                nc.sync.dma_start(blob[:], blob_d[:])
                make_identity(nc, ident[:])
                nc.vector.tensor_copy(hbuf[0][:], h0)   # f32 -> bf16 cast
                nc.vector.tensor_copy(cbuf[0][:], c0)

                # ---- phase A: Xpre GEMM ----
                for m in range(16):
                    g_, b_ = divmod(m, 4)
                    for tck in range(4):
                        ps = psx.tile([128, 512], F32, tag="psx")
                        for e in range(5):
                            nc.tensor.matmul(
                                ps[:],
                                wihT[:, e, ts(m, 128)],
                                xT[:, e, ts(tck, 512)],
                                start=(e == 0),
                                stop=(e == 4),
                            )
                        nc.vector.tensor_copy(
                            xp[:, ts(tck, 512), g_, b_], ps[:]
                        )

          # ---- phase B: recurrence ----
          with tile.TileContext(nc) as tc:
            with (
                tc.tile_pool(name="work", bufs=4) as wpool,
                tc.tile_pool(name="psg", bufs=4, space="PSUM") as psg,
            ):
                with tc.For_i(0, T, U, hint_engines=(mybir.EngineType.PE,),
                              staggered_reset=True) as iv:
                    for u in range(U):
                        toff = nc.snap(iv + u)
                        hcur = hbuf[u % 2]
                        hnxt = hbuf[(u + 1) % 2]
                        ccur = cbuf[u % 2]
                        cnxt = cbuf[(u + 1) % 2]
                        # full-bank tiles: pgA/pgB in separate PSUM banks so
                        # each half's readiness sem fires independently
                        pgAf = psg.tile([128, 512], F32, tag="pgA")
                        pgBf = psg.tile([128, 512], F32, tag="pgB")
                        pgA = pgAf[:, 0:8].rearrange("p (g b) -> p g b", g=4)
                        pgB = pgBf[:, 0:8].rearrange("p (g b) -> p g b", g=4)
                        # k=0,1 for all m (needs h blocks 0,1 = half A of t-1)
                        for k in (0, 1):
                            for m in mA + mB:
                                g_, b_ = divmod(m, 4)
                                pg = pgA if b_ < 2 else pgB
                                nc.tensor.matmul(
                                    pg[:, g_, (b_ % 2) : (b_ % 2) + 1],
                                    whhT[:, k, ts(m, 128)],
                                    hcur[:, k : k + 1],
                                    start=(k == 0),
                                    stop=False,
                                    skip_group_check=True,
                                )
                        # half A: k=2,3 + Xpre inject; then half B
                        for half, pg, bx in ((mA, pgA, slice(0, 2)),
                                             (mB, pgB, slice(2, 4))):
                            nc.tensor.matmul(
                                pg[:, :, :],
                                ident[:],
                                xp[:, ds(toff, 1), :, bx].squeeze(1),
                                start=False,
                                stop=False,
                                skip_group_check=True,
                            )
                            for m in half:
                                g_, b_ = divmod(m, 4)
                                for k in (2, 3):
                                    nc.tensor.matmul(
                                        pg[:, g_, (b_ % 2) : (b_ % 2) + 1],
                                        whhT[:, k, ts(m, 128)],
                                        hcur[:, k : k + 1],
                                        start=False,
                                        stop=(k == 3 and m == half[-1]),
                                        skip_group_check=True,
                                    )
                        # tail, emitted in the exact per-engine FIFO order we
                        # want: ACT [sgA, sgB, tcA, tcB]; DVE [qA, c'A, qB,
                        # hA, c'B, hB]; GpSimd [fcA, fcB, hs].
                        # sg rows: 0=sig(i) 1=sig(f) 2=sig(o) 3=sig(2g~)
                        sgA = wpool.tile([128, 4, 2], F32, tag="sgA")
                        nc.scalar.activation(sgA[:], pgA, AF.Sigmoid)
                        fcA = wpool.tile([128, 2], F32, tag="fcA")
                        nc.gpsimd.tensor_mul(fcA[:], sgA[:, 1, :], ccur[:, 0:2])
                        qA = wpool.tile([128, 2], F32, tag="qA")
                        # q = (sig_g - 0.5) * sig_i = i*tanh(g~)/2
                        nc.vector.scalar_tensor_tensor(
                            qA[:], sgA[:, 3, :], 0.5, sgA[:, 0, :],
                            ALU.subtract, ALU.mult)
                        nc.vector.scalar_tensor_tensor(
                            cnxt[:, 0:2], qA[:], 2.0, fcA[:], ALU.mult, ALU.add)
                        sgB = wpool.tile([128, 4, 2], F32, tag="sgB")
                        nc.scalar.activation(sgB[:], pgB, AF.Sigmoid)
                        fcB = wpool.tile([128, 2], F32, tag="fcB")
                        nc.gpsimd.tensor_mul(fcB[:], sgB[:, 1, :], ccur[:, 2:4])
                        tcA = wpool.tile([128, 2], F32, tag="tcA")
                        nc.scalar.activation(tcA[:], cnxt[:, 0:2], AF.Tanh)
                        qB = wpool.tile([128, 2], F32, tag="qB")
                        nc.vector.scalar_tensor_tensor(
                            qB[:], sgB[:, 3, :], 0.5, sgB[:, 0, :],
                            ALU.subtract, ALU.mult)
                        nc.vector.tensor_mul(hnxt[:, 0:2], sgA[:, 2, :], tcA[:])
                        nc.vector.scalar_tensor_tensor(
                            cnxt[:, 2:4], qB[:], 2.0, fcB[:], ALU.mult, ALU.add)
                        tcB = wpool.tile([128, 2], F32, tag="tcB")
                        nc.scalar.activation(tcB[:], cnxt[:, 2:4], AF.Tanh)
                        nc.vector.tensor_mul(hnxt[:, 2:4], sgB[:, 2, :], tcB[:])
                        nc.gpsimd.tensor_copy(
                            hs[:, ds(toff, 1), :].squeeze(1), hnxt[:]
                        )

        # ---- ctx 2: feats GEMM + output (fresh sems: tail drain stays small) ----
        with tile.TileContext(nc) as tc2:
            with (
                tc2.tile_pool(name="fout", bufs=1) as fpool,
                tc2.tile_pool(name="psf", bufs=2, space="PSUM") as psf,
            ):
                fsb = fpool.tile([NT, T], F32, tag="fsb")
                for tck in range(4):
                    pf = psf.tile([NT, 512], F32, tag="pf")
                    for k in range(4):
                        nc.tensor.matmul(
                            pf[:],
                            woutT[:, k, :],
                            hs[:, tck * 512 : (tck + 1) * 512, k],
                            start=(k == 0),
                            stop=(k == 3),
                        )
                    nc.vector.tensor_copy(fsb[:, ts(tck, 512)], pf[:])
                nc.sync.dma_start(featsT_d[:], fsb[:])

    nc.compile()
    return nc


def build_crf_program():
    nc = _new_nc(1)
    ff_d = nc.dram_tensor("ftf", [NT, T], F32, kind="ExternalInput")
    fb_d = nc.dram_tensor("ftb", [NT, T], F32, kind="ExternalInput")
    brep_d = nc.dram_tensor("brep", [128, 16, NT], F32, kind="ExternalInput")
    ta_d = nc.dram_tensor("ta", [128, 125], F32, kind="ExternalInput")
    tb_d = nc.dram_tensor("tb", [128, 125], F32, kind="ExternalInput")
    fv0_d = nc.dram_tensor("fv0r", [1, 25], F32, kind="ExternalInput")
    stp_d = nc.dram_tensor("stpr", [1, 25], F32, kind="ExternalInput")
    out_d = nc.dram_tensor("logz", [1, 1], F32, kind="ExternalOutput")

    with tile.TileContext(nc) as tc:
        with (
            tc.tile_pool(name="c", bufs=1) as cp,
            tc.tile_pool(name="w", bufs=2) as wp,
            tc.tile_pool(name="ps", bufs=2, space="PSUM") as pp,
            tc.tile_pool(name="dr", bufs=1, space="DRAM") as dp,
        ):
            ftf = cp.tile([NT, T], F32)
            nc.sync.dma_start(ftf[:], ff_d[:])
            ftb = cp.tile([NT, T], F32)
            nc.sync.dma_start(ftb[:], fb_d[:])
            brep = cp.tile([128, 16, NT], F32)
            nc.sync.dma_start(brep[:], brep_d[:])
            ta = cp.tile([128, 125], F32)
            nc.sync.dma_start(ta[:], ta_d[:])
            tb = cp.tile([128, 125], F32)
            nc.sync.dma_start(tb[:], tb_d[:])
            fv0r = cp.tile([1, 25], F32)
            nc.sync.dma_start(fv0r[:], fv0_d[:])
            stpr = cp.tile([1, 25], F32)
            nc.sync.dma_start(stpr[:], stp_d[:])

            ident = cp.tile([128, 128], F32, tag="ident")
            make_identity(nc, ident[:])

            # q[p, k, i*5+j] = trans[k,i] + trans[j,k]
            q = cp.tile([128, 5, 25], F32, tag="q")
            nc.vector.tensor_add(
                q[:],
                ta[:].rearrange("p (k x) -> p k x", k=5),
                tb[:].rearrange("p (k x) -> p k x", k=5),
            )

            # F2[p, c, j] = feats[16p + c, j] (both dirs + bias)
            f2 = cp.tile([128, 16, NT], F32, tag="f2")
            for c in range(16):
                pt = pp.tile([128, NT], F32, tag="pt")
                nc.tensor.transpose(pt[:], ftf[:, c::16], ident[0:NT, 0:NT])
                nc.vector.tensor_add(f2[:, c, :], pt[:], brep[:, c, :])
                pt2 = pp.tile([128, NT], F32, tag="pt")
                nc.tensor.transpose(pt2[:], ftb[:, c::16], ident[0:NT, 0:NT])
                nc.vector.tensor_add(f2[:, c, :], f2[:, c, :], pt2[:])

            def lse_k(dst, tsrc, pdim, shape):
                """dst(AP) = logsumexp over innermost k(=5) of tsrc(AP) [pdim, *shape, 5]."""
                mx = wp.tile([pdim] + shape, F32, tag=f"mx{len(shape)}")
                nc.vector.tensor_reduce(mx[:], tsrc, mybir.AxisListType.X, ALU.max)
                mxb = mx[:].unsqueeze(len(shape) + 1).broadcast_to(
                    [pdim] + shape + [5]
                )
                nc.vector.tensor_sub(tsrc, tsrc, mxb)
                nc.scalar.activation(tsrc, tsrc, AF.Exp)
                ssum = wp.tile([pdim] + shape, F32, tag=f"ss{len(shape)}")
                nc.vector.tensor_reduce(ssum[:], tsrc, mybir.AxisListType.X, ALU.add)
                nc.scalar.activation(ssum[:], ssum[:], AF.Ln)
                nc.vector.tensor_add(dst, mx[:], ssum[:])

            # ---- level 0: 2048 A_t -> 1024 products; pair t=(16p+2d, 16p+2d+1) ----
            tstack = wp.tile([128, 8, 25, 5], F32, tag="t0")
            nc.vector.tensor_add(
                tstack[:],
                q[:].rearrange("p k x -> p x k").unsqueeze(1)
                .broadcast_to([128, 8, 25, 5]),
                f2[:, 0::2, :].unsqueeze(2).broadcast_to([128, 8, 25, 5]),
            )
            lvl = cp.tile([128, 8, 25], F32, tag="lvl8")
            lse_k(lvl[:], tstack[:], 128, [8, 25])
            # += f_odd[j] broadcast over i
            nc.vector.tensor_add(
                lvl[:].rearrange("p d (i j) -> p d i j", i=5),
                lvl[:].rearrange("p d (i j) -> p d i j", i=5),
                f2[:, 1::2, :].unsqueeze(2).broadcast_to([128, 8, 5, 5]),
            )

            def pair_level(src, pdim, nd):
                """src[pdim, nd, 25] -> dst[pdim, nd/2, 25]; adjacent pairs.
                tt[p,d,i*5+j,k] = A[p,d,i*5+k] + B[p,d,k*5+j]; built row-by-row
                since DVE APs allow at most 3 free dims."""
                nd2 = nd // 2
                sv = src[:].rearrange("p (d two) x -> p d two x", two=2)
                tt = wp.tile([pdim, nd2, 25, 5], F32, tag=f"tt{nd2}")
                ttv = tt[:].rearrange("p d (i j) k -> p d i j k", i=5)
                bv = (sv[:, :, 1, :].rearrange("p d (k j) -> p d k j", k=5)
                      .rearrange("p d k j -> p d j k"))
                for i in range(5):
                    av = (sv[:, :, 0, i * 5 : (i + 1) * 5]
                          .unsqueeze(2).broadcast_to([pdim, nd2, 5, 5]))
                    nc.vector.tensor_add(ttv[:, :, i, :, :], av, bv)
                dst = cp.tile([pdim, nd2, 25], F32, tag=f"lvl{pdim}_{nd2}")
                lse_k(dst[:], tt[:], pdim, [nd2, 25])
                return dst

            for nd in (8, 4, 2):
                lvl = pair_level(lvl, 128, nd)
            # lvl: [128, 1, 25]

            # repack 8 partitions -> 1 via DRAM roundtrip
            dr1 = dp.tile([128, 25], F32, tag="dr1")
            nc.sync.dma_start(dr1[:], lvl[:].squeeze(1))
            pk = cp.tile([16, 8, 25], F32, tag="pk16")
            nc.sync.dma_start(pk[:], dr1[:].rearrange("(a b) x -> a b x", b=8))
            cur = pk
            for nd in (8, 4, 2):
                cur = pair_level(cur, 16, nd)
            dr2 = dp.tile([16, 25], F32, tag="dr2")
            nc.sync.dma_start(dr2[:], cur[:].squeeze(1))
            pk2 = cp.tile([1, 16, 25], F32, tag="pk2")
            nc.sync.dma_start(pk2[:], dr2[:].rearrange("(a b) x -> a b x", b=16))
            cur = pk2
            for nd in (16, 8, 4, 2):
                cur = pair_level(cur, 1, nd)
            # cur: [1, 1, 25]
            pfin = cp.tile([1, 5, 5], F32, tag="pfin")
            nc.vector.tensor_copy(pfin[:], cur[:].squeeze(1)
                                  .rearrange("p (i j) -> p i j", i=5))
            # logZ = lse over 25 of (fv0[i] + P[i,j] + trans[STOP, j])
            pfl = pfin[:].rearrange("p i j -> p (i j)")
            nc.vector.tensor_add(pfl, pfl, fv0r[:])
            nc.vector.tensor_add(pfl, pfl, stpr[:])
            m2 = wp.tile([1, 1], F32, tag="m2")
            nc.vector.tensor_reduce(m2[:], pfl, mybir.AxisListType.X, ALU.max)
            nc.vector.tensor_sub(pfl, pfl, m2[:].broadcast_to([1, 25]))
            nc.scalar.activation(pfl, pfl, AF.Exp)
            s2 = wp.tile([1, 1], F32, tag="s2")
            nc.vector.tensor_reduce(s2[:], pfl, mybir.AxisListType.X, ALU.add)
            nc.scalar.activation(s2[:], s2[:], AF.Ln)
            res = cp.tile([1, 1], F32, tag="res")
            nc.vector.tensor_add(res[:], s2[:], m2[:])
            nc.sync.dma_start(out_d[:], res[:])

    nc.compile()
    return nc


def _prep_dir(x, w_ih, w_hh, b, h0d, c0d, w_out_half):
    bf = ml_dtypes.bfloat16
    # gate order after PERM: [i, f, o, g~]; g~ rows (last Hh) pre-scaled by 2
    # so tanh(z) = 2*sigmoid(2z)-1 needs only a single sigmoid table hit.
    gsc = np.ones((G, 1), np.float32)
    gsc[3 * Hh :] = 2.0
    xT = _to_tiles(np.concatenate(
        [np.ascontiguousarray(x.T), np.ones((1, T), np.float32),
         np.zeros((127, T), np.float32)], 0), 5, T).astype(bf)
    wihT = _to_tiles(np.concatenate(
        [np.ascontiguousarray((gsc * w_ih[PERM]).T),
         (gsc[:, 0] * b[PERM])[None, :].astype(np.float32),
         np.zeros((127, G), np.float32)], 0), 5, G).astype(bf)
    whhT = _to_tiles(np.ascontiguousarray((gsc * w_hh[PERM]).T), 4, G).astype(bf)
    woutT = _to_tiles(np.ascontiguousarray(w_out_half.T), 4, NT).astype(bf)
    hc0 = np.stack([h0d.reshape(4, 128).T, c0d.reshape(4, 128).T], 1)
    hc0_bits = np.ascontiguousarray(hc0.astype(np.float32)).view(np.uint16)
    blob = np.concatenate(
        [xT.reshape(128, -1).view(np.uint16),
         wihT.reshape(128, -1).view(np.uint16),
         whhT.reshape(128, -1).view(np.uint16),
         woutT.reshape(128, -1).view(np.uint16),
         hc0_bits.reshape(128, 16)], 1)
    return dict(blob=np.ascontiguousarray(blob).view(bf))


def kernel(sentence, emb, w_ih_f, w_hh_f, b_f, w_ih_b, w_hh_b, b_b,
           w_out, b_out, transitions, h0, c0):
    sentence = np.asarray(sentence)
    emb = np.asarray(emb, dtype=np.float32)
    x = emb[sentence.astype(np.int64)]  # [T, E] host gather
    h0 = np.asarray(h0, np.float32)
    c0 = np.asarray(c0, np.float32)
    w_out = np.asarray(w_out, np.float32)

    in_f = _prep_dir(x, np.asarray(w_ih_f, np.float32),
                     np.asarray(w_hh_f, np.float32), np.asarray(b_f, np.float32),
                     h0[0, 0], c0[0, 0], w_out[:, :Hh])
    in_b = _prep_dir(x[::-1], np.asarray(w_ih_b, np.float32),
                     np.asarray(w_hh_b, np.float32), np.asarray(b_b, np.float32),
                     h0[1, 0], c0[1, 0], w_out[:, Hh:])

    nc_a = build_lstm_program()
    res_a = run_bass_kernel_spmd(nc_a, [in_f, in_b], core_ids=[0, 1])
    ftf = res_a.results[0]["featsT"]           # [5, T]
    ftb = res_a.results[1]["featsT"][:, ::-1]  # un-reverse (marshaling)
    LAST_INFO["neff_a_ns"] = res_a.exec_time_ns
    if res_a.instructions_and_trace:
        LAST_INFO["trace_a"] = res_a.instructions_and_trace[1]

    trans = np.asarray(transitions, np.float32)
    b_out = np.asarray(b_out, np.float32)
    k_, i_, j_ = np.meshgrid(np.arange(5), np.arange(5), np.arange(5), indexing="ij")
    ta = trans[k_, i_]  # [k,i,j] = trans[k,i]
    tb = trans[j_, k_]  # [k,i,j] = trans[j,k]
    ta_rep = np.ascontiguousarray(
        np.broadcast_to(ta.reshape(1, 125), (128, 125))).astype(np.float32)
    tb_rep = np.ascontiguousarray(
        np.broadcast_to(tb.reshape(1, 125), (128, 125))).astype(np.float32)
    brep = np.ascontiguousarray(
        np.broadcast_to(b_out[None, None, :], (128, 16, 5))).astype(np.float32)
    fv0 = np.full((NT,), NEG, np.float32)
    fv0[START] = 0.0
    fv0_rep = np.ascontiguousarray(np.repeat(fv0, 5)[None, :]).astype(np.float32)
    stp_rep = np.ascontiguousarray(np.tile(trans[STOP], 5)[None, :]).astype(np.float32)

    nc_b = build_crf_program()
    in_crf = dict(ftf=np.ascontiguousarray(ftf).astype(np.float32),
                  ftb=np.ascontiguousarray(ftb).astype(np.float32),
                  brep=brep, ta=ta_rep, tb=tb_rep, fv0r=fv0_rep, stpr=stp_rep)
    res_b = run_bass_kernel_spmd(nc_b, [in_crf], core_ids=[0])
    LAST_INFO["neff_b_ns"] = res_b.exec_time_ns
    if res_b.instructions_and_trace:
        LAST_INFO["trace_b"] = res_b.instructions_and_trace[1]
    out = res_b.results[0]["logz"].reshape(())
    return np.asarray(out, dtype=np.float32).reshape(())


# revision 17
# speedup vs baseline: 1.3118x; 1.0070x over previous
"""BiLSTM-CRF Trainium2 kernel (nn_BiLSTM_CRF_44435731645126).

Strategy:
  host: gather x = emb[sentence] (avoids shipping the 205MB table) plus pure
        marshaling (transpose/permute/cast/flip) of weights.
  NEFF A (SPMD, cores 0-1): core0 = forward LSTM, core1 = backward LSTM on
        time-reversed input. Per core: Xpre = x@w_ih.T+b GEMM (bf16),
        2048-step recurrence (64 LDW+MM pairs/step at the ~27ns MM dispatch
        floor), partial featsT = w_out_half @ hs GEMM.
  host: flip backward partial feats (marshaling only).
  NEFF B (1 core): featsT_f + featsT_b + b_out -> CRF forward pass as a
        log-semiring scan tree -> logZ scalar.

Recurrence critical path design (v3):
  - per-half PSUM gate tiles pgA/pgB: half-A tail starts after its 32 MMs.
  - Xpre injected into PSUM by one identity-stationary matmul per half
    (rhs = xp[:, t] dynamic slice), so ACT reads gates straight from PSUM.
  - tanh(g~) computed as 2*sigmoid(2x)-1: g~ rows of W_hh/W_ih/b are
    pre-scaled by 2 on host; ONE sigmoid covers all 4 gates of a half.
    Reconstruction is fused into scalar_tensor_tensor ops:
      q  = (sig_g - 0.5) * sig_i          # = i*tanh(g)/2
      c' = (q * 2) + f*c
  - f*c and the hs spill run on GpSimd (otherwise idle).
"""

import os
import numpy as np
import ml_dtypes

import concourse.bass as bass
from concourse import bacc
import concourse.mybir as mybir
import concourse.tile as tile
from concourse.bass import ds, ts
from concourse.bass_utils import run_bass_kernel_spmd
from concourse.masks import make_identity

F32 = mybir.dt.float32
BF16 = mybir.dt.bfloat16
AF = mybir.ActivationFunctionType
ALU = mybir.AluOpType

T = 2048
E = 512
Hh = 512
G = 2048  # 4*Hh
NT = 5
START, STOP = 3, 4
NEG = -10000.0

U = int(os.environ.get("LSTM_UNROLL", "8"))  # steps per For_i iteration

LAST_INFO = {}

# m-column layout: m = g*4 + b, gate order [i, f, o, g~], b = hidden block.
PERM = np.concatenate([
    np.arange(0, 512),       # i
    np.arange(512, 1024),    # f
    np.arange(1536, 2048),   # o
    np.arange(1024, 1536),   # g~
])


def _to_tiles(mat_t, nk, free):
    """mat_t: [nk*128, free] -> [128, nk, free] with [p, k, f] = mat_t[128k+p, f]."""
    return np.ascontiguousarray(mat_t.reshape(nk, 128, free).transpose(1, 0, 2))


def _new_nc(num_devices):
    return bacc.Bacc("TRN2", target_bir_lowering=False, debug=False,
                     num_devices=num_devices)


def build_lstm_program():
    nc = _new_nc(2)
    NBLOB = 5 * T + 5 * G + 4 * G + 4 * NT + 16
    blob_d = nc.dram_tensor("blob", [128, NBLOB], BF16, kind="ExternalInput")
    featsT_d = nc.dram_tensor("featsT", [NT, T], F32, kind="ExternalOutput")

    mA = [g * 4 + b for g in range(4) for b in range(2)]  # hidden blocks 0,1
    mB = [g * 4 + b for g in range(4) for b in (2, 3)]

    with (
        nc.sbuf_tensor([128, NBLOB], BF16) as blob,
        nc.sbuf_tensor([128, T, 4], BF16) as hs,
    ):
        o0 = 5 * T + 5 * G
        whhT = blob[:, o0 : o0 + 4 * G].rearrange("p (k g) -> p k g", k=4)
        o1 = o0 + 4 * G
        woutT = blob[:, o1 : o1 + 4 * NT].rearrange("p (k j) -> p k j", k=4)
        o2 = o1 + 4 * NT
        hc0 = blob[:, o2 : o2 + 16].bitcast(F32).rearrange(
            "p (two k) -> p two k", two=2)
        h0 = hc0[:, 0, :]
        c0 = hc0[:, 1, :]
        xT = blob[:, : 5 * T].rearrange("p (k t) -> p k t", k=5)
        wihT = blob[:, 5 * T : 5 * T + 5 * G].rearrange("p (k g) -> p k g", k=5)

        with (
            nc.sbuf_tensor([128, 4, 4, T], BF16) as xp,
            nc.sbuf_tensor([128, 128], BF16) as ident,
            nc.sbuf_tensor([128, 4], BF16) as hb0,
            nc.sbuf_tensor([128, 4], BF16) as hb1,
            nc.sbuf_tensor([128, 4], F32) as cb0,
            nc.sbuf_tensor([128, 4], F32) as cb1,
        ):
          hbuf = [hb0, hb1]
          cbuf = [cb0, cb1]
          with tile.TileContext(nc) as tc0:
            with tc0.tile_pool(name="psx", bufs=4, space="PSUM") as psx:
                nc.sync.dma_start(blob[:], blob_d[:])
                make_identity(nc, ident[:])
                nc.vector.tensor_copy(hbuf[0][:], h0)   # f32 -> bf16 cast
                nc.vector.tensor_copy(cbuf[0][:], c0)

                # ---- phase A: Xpre GEMM ----
                for m in range(16):
                    g_, b_ = divmod(m, 4)
                    for tck in range(4):
                        ps = psx.tile([128, 512], F32, tag="psx")
                        for e in range(5):
                            nc.tensor.matmul(
                                ps[:],
                                wihT[:, e, ts(m, 128)],
                                xT[:, e, ts(tck, 512)],
                                start=(e == 0),
                                stop=(e == 4),
                            )
                        nc.vector.tensor_copy(
                            xp[:, g_, b_, ts(tck, 512)], ps[:]
                        )

          # ---- phase B: recurrence ----
          with tile.TileContext(nc) as tc:
            with (
                tc.tile_pool(name="work", bufs=4) as wpool,
                tc.tile_pool(name="psgA", bufs=2, space="PSUM") as psgA,
                tc.tile_pool(name="psgB", bufs=2, space="PSUM") as psgB,
            ):
                with tc.For_i(0, T, U, hint_engines=(mybir.EngineType.PE,),
                              staggered_reset=True) as iv:
                    for u in range(U):
                        toff = nc.snap(iv + u)
                        hcur = hbuf[u % 2]
                        hnxt = hbuf[(u + 1) % 2]
                        ccur = cbuf[u % 2]
                        cnxt = cbuf[(u + 1) % 2]
                        # full-bank tiles: pgA/pgB in separate PSUM banks so
                        # each half's readiness sem fires independently
                        pgAf = psgA.tile([128, 512], F32, tag="pgA")
                        pgBf = psgB.tile([128, 512], F32, tag="pgB")
                        pgA = pgAf[:, 0:8].rearrange("p (g b) -> p g b", g=4)
                        pgB = pgBf[:, 0:8].rearrange("p (g b) -> p g b", g=4)
                        # k=0,1 for all m (needs h blocks 0,1 = half A of t-1)
                        for k in (0, 1):
                            for m in mA + mB:
                                g_, b_ = divmod(m, 4)
                                pg = pgA if b_ < 2 else pgB
                                nc.tensor.matmul(
                                    pg[:, g_, (b_ % 2) : (b_ % 2) + 1],
                                    whhT[:, k, ts(m, 128)],
                                    hcur[:, k : k + 1],
                                    start=(k == 0),
                                    stop=False,
                                    skip_group_check=True,
                                )
                        # half A: k=2,3 + Xpre inject; then half B
                        for half, pg, bx in ((mA, pgA, slice(0, 2)),
                                             (mB, pgB, slice(2, 4))):
                            nc.tensor.matmul(
                                pg[:, :, :],
                                ident[:],
                                xp[:, :, bx, ds(toff, 1)].squeeze(3),
                                start=False,
                                stop=False,
                                skip_group_check=True,
                            )
                            for m in half:
                                g_, b_ = divmod(m, 4)
                                for k in (2, 3):
                                    nc.tensor.matmul(
                                        pg[:, g_, (b_ % 2) : (b_ % 2) + 1],
                                        whhT[:, k, ts(m, 128)],
                                        hcur[:, k : k + 1],
                                        start=False,
                                        stop=(k == 3 and m == half[-1]),
                                        skip_group_check=True,
                                    )
                        # tail, emitted in the exact per-engine FIFO order we
                        # want: ACT [sgA, sgB, tcA, tcB]; DVE [qA, c'A, qB,
                        # hA, c'B, hB]; GpSimd [fcA, fcB, hs].
                        # sg rows: 0=sig(i) 1=sig(f) 2=sig(o) 3=sig(2g~)
                        sgA = wpool.tile([128, 4, 2], F32, tag="sgA")
                        nc.scalar.activation(sgA[:], pgA, AF.Sigmoid)
                        fcA = wpool.tile([128, 2], F32, tag="fcA")
                        nc.gpsimd.tensor_mul(fcA[:], sgA[:, 1, :], ccur[:, 0:2])
                        qA = wpool.tile([128, 2], F32, tag="qA")
                        # q = (sig_g - 0.5) * sig_i = i*tanh(g~)/2
                        nc.vector.scalar_tensor_tensor(
                            qA[:], sgA[:, 3, :], 0.5, sgA[:, 0, :],
                            ALU.subtract, ALU.mult)
                        nc.vector.scalar_tensor_tensor(
                            cnxt[:, 0:2], qA[:], 2.0, fcA[:], ALU.mult, ALU.add)
                        sgB = wpool.tile([128, 4, 2], F32, tag="sgB")
                        nc.scalar.activation(sgB[:], pgB, AF.Sigmoid)
                        fcB = wpool.tile([128, 2], F32, tag="fcB")
                        nc.gpsimd.tensor_mul(fcB[:], sgB[:, 1, :], ccur[:, 2:4])
                        tcA = wpool.tile([128, 2], F32, tag="tcA")
                        nc.scalar.activation(tcA[:], cnxt[:, 0:2], AF.Tanh)
                        qB = wpool.tile([128, 2], F32, tag="qB")
                        nc.vector.scalar_tensor_tensor(
                            qB[:], sgB[:, 3, :], 0.5, sgB[:, 0, :],
                            ALU.subtract, ALU.mult)
                        nc.gpsimd.tensor_mul(hnxt[:, 0:2], sgA[:, 2, :], tcA[:])
                        nc.vector.scalar_tensor_tensor(
                            cnxt[:, 2:4], qB[:], 2.0, fcB[:], ALU.mult, ALU.add)
                        tcB = wpool.tile([128, 2], F32, tag="tcB")
                        nc.scalar.activation(tcB[:], cnxt[:, 2:4], AF.Tanh)
                        nc.vector.tensor_mul(hnxt[:, 2:4], sgB[:, 2, :], tcB[:])
                        nc.gpsimd.tensor_copy(
                            hs[:, ds(toff, 1), :].squeeze(1), hnxt[:]
                        )

        # ---- ctx 2: feats GEMM + output (fresh sems: tail drain stays small) ----
        with tile.TileContext(nc) as tc2:
            with (
                tc2.tile_pool(name="fout", bufs=1) as fpool,
                tc2.tile_pool(name="psf", bufs=2, space="PSUM") as psf,
            ):
                fsb = fpool.tile([NT, T], F32, tag="fsb")
                for tck in range(4):
                    pf = psf.tile([NT, 512], F32, tag="pf")
                    for k in range(4):
                        nc.tensor.matmul(
                            pf[:],
                            woutT[:, k, :],
                            hs[:, tck * 512 : (tck + 1) * 512, k],
                            start=(k == 0),
                            stop=(k == 3),
                        )
                    nc.vector.tensor_copy(fsb[:, ts(tck, 512)], pf[:])
                nc.sync.dma_start(featsT_d[:], fsb[:])

    nc.compile()
    return nc


def build_crf_program():
    nc = _new_nc(1)
    ff_d = nc.dram_tensor("ftf", [NT, T], F32, kind="ExternalInput")
    fb_d = nc.dram_tensor("ftb", [NT, T], F32, kind="ExternalInput")
    brep_d = nc.dram_tensor("brep", [128, 16, NT], F32, kind="ExternalInput")
    ta_d = nc.dram_tensor("ta", [128, 125], F32, kind="ExternalInput")
    tb_d = nc.dram_tensor("tb", [128, 125], F32, kind="ExternalInput")
    fv0_d = nc.dram_tensor("fv0r", [1, 25], F32, kind="ExternalInput")
    stp_d = nc.dram_tensor("stpr", [1, 25], F32, kind="ExternalInput")
    out_d = nc.dram_tensor("logz", [1, 1], F32, kind="ExternalOutput")

    with tile.TileContext(nc) as tc:
        with (
            tc.tile_pool(name="c", bufs=1) as cp,
            tc.tile_pool(name="w", bufs=2) as wp,
            tc.tile_pool(name="ps", bufs=2, space="PSUM") as pp,
            tc.tile_pool(name="dr", bufs=1, space="DRAM") as dp,
        ):
            ftf = cp.tile([NT, T], F32)
            nc.sync.dma_start(ftf[:], ff_d[:])
            ftb = cp.tile([NT, T], F32)
            nc.sync.dma_start(ftb[:], fb_d[:])
            brep = cp.tile([128, 16, NT], F32)
            nc.sync.dma_start(brep[:], brep_d[:])
            ta = cp.tile([128, 125], F32)
            nc.sync.dma_start(ta[:], ta_d[:])
            tb = cp.tile([128, 125], F32)
            nc.sync.dma_start(tb[:], tb_d[:])
            fv0r = cp.tile([1, 25], F32)
            nc.sync.dma_start(fv0r[:], fv0_d[:])
            stpr = cp.tile([1, 25], F32)
            nc.sync.dma_start(stpr[:], stp_d[:])

            ident = cp.tile([128, 128], F32, tag="ident")
            make_identity(nc, ident[:])

            # q[p, k, i*5+j] = trans[k,i] + trans[j,k]
            q = cp.tile([128, 5, 25], F32, tag="q")
            nc.vector.tensor_add(
                q[:],
                ta[:].rearrange("p (k x) -> p k x", k=5),
                tb[:].rearrange("p (k x) -> p k x", k=5),
            )

            # F2[p, c, j] = feats[16p + c, j] (both dirs + bias)
            f2 = cp.tile([128, 16, NT], F32, tag="f2")
            for c in range(16):
                pt = pp.tile([128, NT], F32, tag="pt")
                nc.tensor.transpose(pt[:], ftf[:, c::16], ident[0:NT, 0:NT])
                nc.vector.tensor_add(f2[:, c, :], pt[:], brep[:, c, :])
                pt2 = pp.tile([128, NT], F32, tag="pt")
                nc.tensor.transpose(pt2[:], ftb[:, c::16], ident[0:NT, 0:NT])
                nc.vector.tensor_add(f2[:, c, :], f2[:, c, :], pt2[:])

            def lse_k(dst, tsrc, pdim, shape):
                """dst(AP) = logsumexp over innermost k(=5) of tsrc(AP) [pdim, *shape, 5]."""
                mx = wp.tile([pdim] + shape, F32, tag=f"mx{len(shape)}")
                nc.vector.tensor_reduce(mx[:], tsrc, mybir.AxisListType.X, ALU.max)
                mxb = mx[:].unsqueeze(len(shape) + 1).broadcast_to(
                    [pdim] + shape + [5]
                )
                nc.vector.tensor_sub(tsrc, tsrc, mxb)
                nc.scalar.activation(tsrc, tsrc, AF.Exp)
                ssum = wp.tile([pdim] + shape, F32, tag=f"ss{len(shape)}")
                nc.vector.tensor_reduce(ssum[:], tsrc, mybir.AxisListType.X, ALU.add)
                nc.scalar.activation(ssum[:], ssum[:], AF.Ln)
                nc.vector.tensor_add(dst, mx[:], ssum[:])

            # ---- level 0: 2048 A_t -> 1024 products; pair t=(16p+2d, 16p+2d+1) ----
            tstack = wp.tile([128, 8, 25, 5], F32, tag="t0")
            nc.vector.tensor_add(
                tstack[:],
                q[:].rearrange("p k x -> p x k").unsqueeze(1)
                .broadcast_to([128, 8, 25, 5]),
                f2[:, 0::2, :].unsqueeze(2).broadcast_to([128, 8, 25, 5]),
            )
            lvl = cp.tile([128, 8, 25], F32, tag="lvl8")
            lse_k(lvl[:], tstack[:], 128, [8, 25])
            # += f_odd[j] broadcast over i
            nc.vector.tensor_add(
                lvl[:].rearrange("p d (i j) -> p d i j", i=5),
                lvl[:].rearrange("p d (i j) -> p d i j", i=5),
                f2[:, 1::2, :].unsqueeze(2).broadcast_to([128, 8, 5, 5]),
            )

            def pair_level(src, pdim, nd):
                """src[pdim, nd, 25] -> dst[pdim, nd/2, 25]; adjacent pairs.
                tt[p,d,i*5+j,k] = A[p,d,i*5+k] + B[p,d,k*5+j]; built row-by-row
                since DVE APs allow at most 3 free dims."""
                nd2 = nd // 2
                sv = src[:].rearrange("p (d two) x -> p d two x", two=2)
                tt = wp.tile([pdim, nd2, 25, 5], F32, tag=f"tt{nd2}")
                ttv = tt[:].rearrange("p d (i j) k -> p d i j k", i=5)
                bv = (sv[:, :, 1, :].rearrange("p d (k j) -> p d k j", k=5)
                      .rearrange("p d k j -> p d j k"))
                for i in range(5):
                    av = (sv[:, :, 0, i * 5 : (i + 1) * 5]
                          .unsqueeze(2).broadcast_to([pdim, nd2, 5, 5]))
                    nc.vector.tensor_add(ttv[:, :, i, :, :], av, bv)
                dst = cp.tile([pdim, nd2, 25], F32, tag=f"lvl{pdim}_{nd2}")
                lse_k(dst[:], tt[:], pdim, [nd2, 25])
                return dst

            for nd in (8, 4, 2):
                lvl = pair_level(lvl, 128, nd)
            # lvl: [128, 1, 25]

            # repack 8 partitions -> 1 via DRAM roundtrip
            dr1 = dp.tile([128, 25], F32, tag="dr1")
            nc.sync.dma_start(dr1[:], lvl[:].squeeze(1))
            pk = cp.tile([16, 8, 25], F32, tag="pk16")
            nc.sync.dma_start(pk[:], dr1[:].rearrange("(a b) x -> a b x", b=8))
            cur = pk
            for nd in (8, 4, 2):
                cur = pair_level(cur, 16, nd)
            dr2 = dp.tile([16, 25], F32, tag="dr2")
            nc.sync.dma_start(dr2[:], cur[:].squeeze(1))
            pk2 = cp.tile([1, 16, 25], F32, tag="pk2")
            nc.sync.dma_start(pk2[:], dr2[:].rearrange("(a b) x -> a b x", b=16))
            cur = pk2
            for nd in (16, 8, 4, 2):
                cur = pair_level(cur, 1, nd)
            # cur: [1, 1, 25]
            pfin = cp.tile([1, 5, 5], F32, tag="pfin")
            nc.vector.tensor_copy(pfin[:], cur[:].squeeze(1)
                                  .rearrange("p (i j) -> p i j", i=5))
            # logZ = lse over 25 of (fv0[i] + P[i,j] + trans[STOP, j])
            pfl = pfin[:].rearrange("p i j -> p (i j)")
            nc.vector.tensor_add(pfl, pfl, fv0r[:])
            nc.vector.tensor_add(pfl, pfl, stpr[:])
            m2 = wp.tile([1, 1], F32, tag="m2")
            nc.vector.tensor_reduce(m2[:], pfl, mybir.AxisListType.X, ALU.max)
            nc.vector.tensor_sub(pfl, pfl, m2[:].broadcast_to([1, 25]))
            nc.scalar.activation(pfl, pfl, AF.Exp)
            s2 = wp.tile([1, 1], F32, tag="s2")
            nc.vector.tensor_reduce(s2[:], pfl, mybir.AxisListType.X, ALU.add)
            nc.scalar.activation(s2[:], s2[:], AF.Ln)
            res = cp.tile([1, 1], F32, tag="res")
            nc.vector.tensor_add(res[:], s2[:], m2[:])
            nc.sync.dma_start(out_d[:], res[:])

    nc.compile()
    return nc


def _prep_dir(x, w_ih, w_hh, b, h0d, c0d, w_out_half):
    bf = ml_dtypes.bfloat16
    # gate order after PERM: [i, f, o, g~]; g~ rows (last Hh) pre-scaled by 2
    # so tanh(z) = 2*sigmoid(2z)-1 needs only a single sigmoid table hit.
    gsc = np.ones((G, 1), np.float32)
    gsc[3 * Hh :] = 2.0
    xT = _to_tiles(np.concatenate(
        [np.ascontiguousarray(x.T), np.ones((1, T), np.float32),
         np.zeros((127, T), np.float32)], 0), 5, T).astype(bf)
    wihT = _to_tiles(np.concatenate(
        [np.ascontiguousarray((gsc * w_ih[PERM]).T),
         (gsc[:, 0] * b[PERM])[None, :].astype(np.float32),
         np.zeros((127, G), np.float32)], 0), 5, G).astype(bf)
    whhT = _to_tiles(np.ascontiguousarray((gsc * w_hh[PERM]).T), 4, G).astype(bf)
    woutT = _to_tiles(np.ascontiguousarray(w_out_half.T), 4, NT).astype(bf)
    hc0 = np.stack([h0d.reshape(4, 128).T, c0d.reshape(4, 128).T], 1)
    hc0_bits = np.ascontiguousarray(hc0.astype(np.float32)).view(np.uint16)
    blob = np.concatenate(
        [xT.reshape(128, -1).view(np.uint16),
         wihT.reshape(128, -1).view(np.uint16),
         whhT.reshape(128, -1).view(np.uint16),
         woutT.reshape(128, -1).view(np.uint16),
         hc0_bits.reshape(128, 16)], 1)
    return dict(blob=np.ascontiguousarray(blob).view(bf))


def kernel(sentence, emb, w_ih_f, w_hh_f, b_f, w_ih_b, w_hh_b, b_b,
           w_out, b_out, transitions, h0, c0):
    sentence = np.asarray(sentence)
    emb = np.asarray(emb, dtype=np.float32)
    x = emb[sentence.astype(np.int64)]  # [T, E] host gather
    h0 = np.asarray(h0, np.float32)
    c0 = np.asarray(c0, np.float32)
    w_out = np.asarray(w_out, np.float32)

    in_f = _prep_dir(x, np.asarray(w_ih_f, np.float32),
                     np.asarray(w_hh_f, np.float32), np.asarray(b_f, np.float32),
                     h0[0, 0], c0[0, 0], w_out[:, :Hh])
    in_b = _prep_dir(x[::-1], np.asarray(w_ih_b, np.float32),
                     np.asarray(w_hh_b, np.float32), np.asarray(b_b, np.float32),
                     h0[1, 0], c0[1, 0], w_out[:, Hh:])

    nc_a = build_lstm_program()
    res_a = run_bass_kernel_spmd(nc_a, [in_f, in_b], core_ids=[0, 1])
    ftf = res_a.results[0]["featsT"]           # [5, T]
    ftb = res_a.results[1]["featsT"][:, ::-1]  # un-reverse (marshaling)
    LAST_INFO["neff_a_ns"] = res_a.exec_time_ns
    if res_a.instructions_and_trace:
        LAST_INFO["trace_a"] = res_a.instructions_and_trace[1]

    trans = np.asarray(transitions, np.float32)
    b_out = np.asarray(b_out, np.float32)
    k_, i_, j_ = np.meshgrid(np.arange(5), np.arange(5), np.arange(5), indexing="ij")
    ta = trans[k_, i_]  # [k,i,j] = trans[k,i]
    tb = trans[j_, k_]  # [k,i,j] = trans[j,k]
    ta_rep = np.ascontiguousarray(
        np.broadcast_to(ta.reshape(1, 125), (128, 125))).astype(np.float32)
    tb_rep = np.ascontiguousarray(
        np.broadcast_to(tb.reshape(1, 125), (128, 125))).astype(np.float32)
    brep = np.ascontiguousarray(
        np.broadcast_to(b_out[None, None, :], (128, 16, 5))).astype(np.float32)
    fv0 = np.full((NT,), NEG, np.float32)
    fv0[START] = 0.0
    fv0_rep = np.ascontiguousarray(np.repeat(fv0, 5)[None, :]).astype(np.float32)
    stp_rep = np.ascontiguousarray(np.tile(trans[STOP], 5)[None, :]).astype(np.float32)

    nc_b = build_crf_program()
    in_crf = dict(ftf=np.ascontiguousarray(ftf).astype(np.float32),
                  ftb=np.ascontiguousarray(ftb).astype(np.float32),
                  brep=brep, ta=ta_rep, tb=tb_rep, fv0r=fv0_rep, stpr=stp_rep)
    res_b = run_bass_kernel_spmd(nc_b, [in_crf], core_ids=[0])
    LAST_INFO["neff_b_ns"] = res_b.exec_time_ns
    if res_b.instructions_and_trace:
        LAST_INFO["trace_b"] = res_b.instructions_and_trace[1]
    out = res_b.results[0]["logz"].reshape(())
    return np.asarray(out, dtype=np.float32).reshape(())
